# revision 31
# baseline (speedup 1.0000x reference)
"""Trainium2 Bass kernel for nn_ClassModel_72318659330833.

Strategy:
  - Data-parallel over batch: 8 cores x 32 rows.
  - Host-side prep (cheap, index/layout only): find mask positions, gather
    mask hidden states + pooler rows, build additive softmax mask, transpose
    weights into lhsT layouts, slice the 30522-row decoder down to the 21
    label rows actually used by the output.
  - Device per core: attention scores via PE (f32, d-on-partition X^T),
    masked softmax (DVE+ACT), attention-weighted sum via PE (bf16, s-on-
    partition X), dense tanh layer via PE (bf16, output directly in
    transposed layout), 21-row decoder + label-word mixing + sentiment head.
"""

import numpy as np
import ml_dtypes

import concourse.bacc as bacc
import concourse.bass as bass
import concourse.tile as tile
from concourse import mybir
from concourse import bass_utils
from concourse.masks import make_identity

BF16 = ml_dtypes.bfloat16

B, S, D, V = 256, 128, 768, 30522
MASK_ID = 103
NCORES = 8
BC = B // NCORES          # 32 rows per core
L = 104                   # max attention window length (length < 105)
C6 = D // 128             # 6 chunks of 128 along D
C12 = 2 * D // 128        # 12 chunks along 2D

LABEL_IDS = [
    [2307, 2204, 3835, 2157, 6581, 2986, 5151, 3893],
    [7929, 24791, 8699, 4257, 16021, 6623],
    [6659, 2919, 11771, 3532, 11325, 4997, 13135],
]
FLAT_IDS = [i for g in LABEL_IDS for i in g]  # 21 ids
NK = len(FLAT_IDS)

PF = 410 + L              # packed f32 constants width
PB = 192 + 6 * NK         # packed bf16 constants width

KERNEL_TRACE = False      # test.py sets True to capture NTFF exec time
LAST_RESULT = None        # bass_utils.BassKernelResults of last run

_compiled = None


def _body(nc, tc, t):
    """Emit the per-core kernel body. t: dict name->dram AP."""
    f32 = mybir.dt.float32
    bf16 = mybir.dt.bfloat16

    from contextlib import ExitStack

    ctx = ExitStack()
    singles = ctx.enter_context(tc.tile_pool(name="singles", bufs=1))
    work = ctx.enter_context(tc.tile_pool(name="work", bufs=1))
    psum = ctx.enter_context(tc.tile_pool(name="psum", bufs=1, space="PSUM"))

    # ---- resident SBUF tiles ---------------------------------------------
    # Big tensors get their own DMA, split across the two HWDGE queues
    # (sync + scalar); all small constants arrive in two packed buffers.
    xt_sb = singles.tile([128, C6, BC * L], f32, tag="xt")
    nc.sync.dma_start(out=xt_sb[:], in_=t["xt"][:])
    packf_sb = singles.tile([128, PF], f32, tag="packf")
    nc.sync.dma_start(out=packf_sb[:], in_=t["packf"][:])
    packb_sb = singles.tile([128, PB], bf16, tag="packb")
    nc.sync.dma_start(out=packb_sb[:], in_=t["packb"][:])
    x_sb = singles.tile([L, BC, D], bf16, tag="x")
    nc.gpsimd.dma_start(out=x_sb[:], in_=t["x"][:])
    wdt_sb = singles.tile([128, C12, D], bf16, tag="wdt")
    nc.gpsimd.dma_start(out=wdt_sb[:], in_=t["wdt"][:])

    mt_sb = packf_sb[:, 0:192].rearrange("p (c b) -> p c b", c=C6)
    poolert_sb = packf_sb[:, 192:384].rearrange("p (c b) -> p c b", c=C6)
    sentit_sb = packf_sb[:, 384:396].rearrange("p (c b) -> p c b", c=C6)
    db_sb = packf_sb[:, 396:402]
    wf_sb = packf_sb[0:NK, 402:408]
    dbdec_sb = packf_sb[0:NK, 408:409]
    sb_sb = packf_sb[0:2, 409:410]
    amask_sb = packf_sb[0:BC, 410 : 410 + L]
    mtb_sb = packb_sb[:, 0:192].rearrange("p (c b) -> p c b", c=C6)
    dect_sb = packb_sb[:, 192 : 192 + C6 * NK].rearrange("p (c k) -> p c k", c=C6)

    # diagonal-block selector mask: dmask[b, q, i, s] = 1.0 iff b == q*8 + i
    dmask_sb = singles.tile([BC, BC // 8, 8, L], f32, tag="dmask")
    nc.sync.dma_start(out=dmask_sb[:], in_=t["dmask"][:])

    ident_f = singles.tile([BC, BC], f32, tag="identf")
    make_identity(nc, ident_f[:])

    # ---- 1+2. attention scores via all-pairs matmuls ---------------------
    # S_all[b', (b,s)] = m_b' . X_b[s]; diagonal blocks b'==b are the real
    # scores. Processed in 4 quarters of 8 rows (2 PSUM banks each, double
    # buffered) with per-row diagonal extraction on DVE.
    QB = 8                                        # b-rows per quarter
    NQ = BC // QB                                 # 4 quarters
    QN = QB * L                                   # 832 free elems
    reds = []
    for q in range(NQ):
        ps_q = psum.tile([BC, QN], f32, tag="big", bufs=2)
        for (j0, jn) in ((0, 512), (512, QN - 512)):
            for c in range(C6):
                nc.tensor.matmul(
                    ps_q[:, j0 : j0 + jn],
                    mt_sb[:, c, :],              # lhsT [128, 32]
                    xt_sb[:, c, q * QN + j0 : q * QN + j0 + jn],
                    start=(c == 0),
                    stop=(c == C6 - 1),
                )
        # zero out the off-diagonal b-blocks, then reduce over the block idx
        masked = work.tile([BC, QB, L], f32, tag="masked", bufs=2)
        nc.vector.tensor_tensor(
            out=masked[:],
            in0=ps_q[:].rearrange("p (i s) -> p i s", i=QB),
            in1=dmask_sb[:, q, :, :],
            op=mybir.AluOpType.mult,
        )
        red = work.tile([BC, L], f32, tag="red", bufs=NQ)
        nc.vector.tensor_reduce(
            out=red[:],
            in_=masked[:].rearrange("p i s -> p s i"),
            axis=mybir.AxisListType.X,
            op=mybir.AluOpType.add,
        )
        reds.append(red)
    scores_sb = work.tile([BC, L], f32, tag="scores_sb")
    nc.vector.tensor_tensor(
        out=scores_sb[:], in0=reds[0][:], in1=reds[1][:], op=mybir.AluOpType.add
    )
    nc.vector.tensor_tensor(
        out=reds[2][:], in0=reds[2][:], in1=reds[3][:], op=mybir.AluOpType.add
    )
    nc.vector.tensor_tensor(
        out=scores_sb[:], in0=scores_sb[:], in1=reds[2][:], op=mybir.AluOpType.add
    )
    nc.vector.tensor_tensor(
        out=scores_sb[:], in0=scores_sb[:], in1=amask_sb[:], op=mybir.AluOpType.add
    )
    mx = work.tile([BC, 1], f32, tag="mx")
    nc.vector.tensor_reduce(
        out=mx[:], in_=scores_sb[:], axis=mybir.AxisListType.X, op=mybir.AluOpType.max
    )
    negmax = work.tile([BC, 1], f32, tag="negmax")
    nc.vector.tensor_scalar_mul(negmax[:], mx[:], -1.0)
    p_sb = work.tile([BC, L], f32, tag="p_sb")
    sumexp = work.tile([BC, 1], f32, tag="sumexp")
    nc.scalar.activation(
        out=p_sb[:],
        in_=scores_sb[:],
        func=mybir.ActivationFunctionType.Exp,
        bias=negmax[:],
        scale=1.0,
        accum_out=sumexp[:],
    )
    rsum = work.tile([BC, 1], f32, tag="rsum")
    nc.vector.reciprocal(out=rsum[:], in_=sumexp[:])
    nc.vector.tensor_scalar_mul(p_sb[:], p_sb[:], rsum[:])

    # ---- 3. p^T  (transpose [BC, L] -> [L, BC], cast bf16) ---------------
    ps_pt = psum.tile([L, BC], f32, tag="small")
    nc.tensor.transpose(ps_pt[:], p_sb[:], ident_f[:])
    pt_sb = work.tile([L, BC], bf16, tag="pt_sb")
    nc.vector.tensor_copy(pt_sb[:], ps_pt[:])

    # ---- 4+5. att^T[d, b] = sum_s X[b,s,d] p[b,s], built directly in the
    # transposed layout: per (b, d-chunk) matmul with X slice stationary.
    ps_attt = psum.tile([128, C6, BC], f32, tag="attt")
    for b in range(BC):
        for c in range(C6):
            nc.tensor.matmul(
                ps_attt[:, c, b : b + 1],
                x_sb[:, b, c * 128 : (c + 1) * 128],  # lhsT [L, 128]
                pt_sb[:, b : b + 1],                  # rhs  [L, 1]
                start=True,
                stop=True,
            )
    attt_sb = work.tile([128, C6, BC], bf16, tag="attt_sb")
    nc.vector.tensor_copy(attt_sb[:], ps_attt[:])

    # ---- 6. dense: h^T[o, b] = tanh(sum_i W[o,i] feats[b,i] + db[o]) -----
    ps_ht = psum.tile([128, C6, BC], f32, tag="ht")
    for c in range(C6):
        for k in range(C12):
            rhs = attt_sb[:, k, :] if k < C6 else mtb_sb[:, k - C6, :]
            nc.tensor.matmul(
                ps_ht[:, c, :],
                wdt_sb[:, k, c * 128 : (c + 1) * 128],  # lhsT [128, 128]
                rhs,                                     # rhs  [128, BC]
                start=(k == 0),
                stop=(k == C12 - 1),
            )
    ht_sb = work.tile([128, C6, BC], bf16, tag="ht_sb")
    for c in range(C6):
        nc.scalar.activation(
            out=ht_sb[:, c, :],
            in_=ps_ht[:, c, :],
            func=mybir.ActivationFunctionType.Tanh,
            bias=db_sb[:, c : c + 1],
            scale=1.0,
        )

    # ---- 7. decoder (21 label rows): p21^T[k, b] -------------------------
    ps_p21 = psum.tile([NK, BC], f32, tag="small")
    for c in range(C6):
        nc.tensor.matmul(
            ps_p21[:],
            dect_sb[:, c, :],                    # lhsT [128, 21]
            ht_sb[:, c, :],                      # rhs  [128, BC]
            start=(c == 0),
            stop=(c == C6 - 1),
        )
    p21t_sb = work.tile([NK, BC], f32, tag="p21t_sb")
    nc.scalar.activation(
        out=p21t_sb[:],
        in_=ps_p21[:],
        func=mybir.ActivationFunctionType.Tanh,
        bias=dbdec_sb[:],
        scale=1.0,
    )

    # ---- 8. label mixing: out6[b, j*3+g] ---------------------------------
    ps_out6 = psum.tile([BC, 6], f32, tag="small")
    nc.tensor.matmul(ps_out6[:], p21t_sb[:], wf_sb[:], start=True, stop=True)
    out6_sb = work.tile([BC, 6], f32, tag="out6_sb")
    nc.vector.tensor_copy(out6_sb[:], ps_out6[:])
    nc.sync.dma_start(out=t["out6"][:], in_=out6_sb[:])

    # ---- 9. sentiment head: cat^T[c, b] ----------------------------------
    ps_cat = psum.tile([2, BC], f32, tag="small")
    for c in range(C6):
        nc.tensor.matmul(
            ps_cat[:],
            sentit_sb[:, c, :],                  # lhsT [128, 2]
            poolert_sb[:, c, :],                 # rhs  [128, BC]
            start=(c == 0),
            stop=(c == C6 - 1),
        )
    catt_sb = work.tile([2, BC], f32, tag="catt_sb")
    nc.vector.tensor_scalar_add(catt_sb[:], ps_cat[:], sb_sb[:])
    nc.sync.dma_start(out=t["catt"][:], in_=catt_sb[:])

    ctx.close()


def _build():
    global _compiled
    if _compiled is not None:
        return _compiled
    f32 = mybir.dt.float32
    bf16 = mybir.dt.bfloat16
    nc = bacc.Bacc("TRN2", target_bir_lowering=False, debug=False)
    t = {}

    def din(name, shape, dt):
        t[name] = nc.dram_tensor(name, shape, dt, kind="ExternalInput").ap()

    def dout(name, shape, dt):
        t[name] = nc.dram_tensor(name, shape, dt, kind="ExternalOutput").ap()

    din("x", [L, BC, D], bf16)
    din("xt", [128, C6, BC * L], f32)
    din("wdt", [128, C12, D], bf16)
    din("packf", [128, PF], f32)
    din("packb", [128, PB], bf16)
    din("dmask", [BC, BC // 8, 8, L], f32)
    dout("out6", [BC, 6], f32)
    dout("catt", [2, BC], f32)

    with tile.TileContext(nc) as tc:
        _body(nc, tc, t)
    nc.compile()
    _compiled = nc
    return nc


def _chunkT(a2d):
    """[N, D-like] -> [128, D//128, N] chunked transpose layout."""
    d = a2d.shape[1]
    return np.ascontiguousarray(a2d.T.reshape(d // 128, 128, a2d.shape[0]).transpose(1, 0, 2))


def kernel(**inputs):
    global LAST_RESULT
    bert = np.asarray(inputs["bert_out"], dtype=np.float32)      # [B, S, D]
    ids = np.asarray(inputs["input_ids"])
    length = np.asarray(inputs["length"]).astype(np.int64)
    senti_w = np.asarray(inputs["senti_w"], dtype=np.float32)
    senti_b = np.asarray(inputs["senti_b"], dtype=np.float32)
    dense_w = np.asarray(inputs["dense_w"], dtype=np.float32)    # [D, 2D]
    dense_b = np.asarray(inputs["dense_b"], dtype=np.float32)
    dec_w = np.asarray(inputs["dec_w"], dtype=np.float32)        # [V, D]
    dec_b = np.asarray(inputs["dec_b"], dtype=np.float32)
    w0 = np.asarray(inputs["w0"], dtype=np.float32)
    w1 = np.asarray(inputs["w1"], dtype=np.float32)
    w2 = np.asarray(inputs["w2"], dtype=np.float32)

    mask_pos = np.argmax(ids == MASK_ID, axis=1)                 # [B]
    m = bert[np.arange(B), mask_pos]                             # [B, D]
    pooler = bert[:, 0]                                          # [B, D]
    xs = bert[:, 3 : 3 + L]                                      # [B, L, D]

    # shared weight layouts
    wdt = np.ascontiguousarray(
        dense_w.T.reshape(C12, 128, D).transpose(1, 0, 2)
    ).astype(BF16)                                               # [128, 12, D]
    dec21 = dec_w[FLAT_IDS]                                      # [21, D]
    dect = _chunkT(dec21).astype(BF16)                           # [128, 6, 21]
    sentit = _chunkT(senti_w)                                    # [128, 6, 2]
    wf = np.zeros((NK, 6), np.float32)
    off = 0
    for g, wg in enumerate((w0, w1, w2)):
        k = wg.shape[1]
        for j in range(2):
            wf[off : off + k, j * 3 + g] = wg[j]
        off += k
    db = np.ascontiguousarray(dense_b.reshape(C6, 128).T)        # [128, 6]
    dbdec = dec_b[FLAT_IDS][:, None].astype(np.float32)
    sb = senti_b[:, None]
    dmask = np.zeros((BC, BC // 8, 8, L), np.float32)
    for b in range(BC):
        dmask[b, b // 8, b % 8, :] = 1.0

    nc = _build()

    in_maps = []
    for k in range(NCORES):
        sl = slice(k * BC, (k + 1) * BC)
        xsk = xs[sl]                                             # [BC, L, D]
        x_in = np.ascontiguousarray(xsk.transpose(1, 0, 2)).astype(BF16)
        xt_in = np.ascontiguousarray(
            xsk.transpose(2, 0, 1).reshape(C6, 128, BC, L).transpose(1, 0, 2, 3)
        ).reshape(128, C6, BC * L)                               # [128, 6, 3328] f32
        mtk = _chunkT(m[sl])                                     # [128, 6, BC]
        amask = np.where(
            np.arange(L)[None, :] < length[sl, None], 0.0, -1e30
        ).astype(np.float32)
        packf = np.zeros((128, PF), np.float32)
        packf[:, 0:192] = mtk.reshape(128, 192)
        packf[:, 192:384] = _chunkT(pooler[sl]).reshape(128, 192)
        packf[:, 384:396] = sentit.reshape(128, 12)
        packf[:, 396:402] = db
        packf[0:NK, 402:408] = wf
        packf[0:NK, 408:409] = dbdec
        packf[0:2, 409:410] = sb
        packf[0:BC, 410 : 410 + L] = amask
        packb = np.zeros((128, PB), BF16)
        packb[:, 0:192] = mtk.astype(BF16).reshape(128, 192)
        packb[:, 192:PB] = dect.reshape(128, C6 * NK)
        in_maps.append(
            {
                "x": x_in,
                "xt": xt_in,
                "wdt": wdt,
                "packf": packf,
                "packb": packb,
                "dmask": dmask,
            }
        )

    res = bass_utils.run_bass_kernel_spmd(
        nc, in_maps, core_ids=list(range(NCORES)), trace=KERNEL_TRACE
    )
    LAST_RESULT = res

    category_out = np.empty((B, 2), np.float32)
    out = np.empty((B, 2, 3), np.float32)
    for k in range(NCORES):
        sl = slice(k * BC, (k + 1) * BC)
        category_out[sl] = res.results[k]["catt"].T
        out[sl] = res.results[k]["out6"].reshape(BC, 2, 3)
    return category_out, out


# revision 32
# speedup vs baseline: 1.4032x; 1.4032x over previous
"""Trainium2 Bass kernel for nn_ClassModel_72318659330833.

Strategy:
  - Data-parallel over batch: 8 cores x 32 rows; the batch is globally
    sorted by attention length and dealt round-robin so all cores share an
    identical slot-length profile (SPMD: one program).
  - Slots are grouped into 4 quarters of 8; each quarter is padded to its
    own max length Lq (multiple of 8). This makes the ragged attention
    window static per quarter while skipping ~35% of bytes and FLOPs.
  - Host-side prep (index/layout only): mask-row gather, transposed/chunked
    weight layouts, additive masks, decoder sliced to the 21 label rows.
  - Device per core, pipelined over quarters: attention scores via PE
    all-pairs (f32 X^T, exact), diagonal extraction via mask-multiply +
    strided reduce (DVE), masked softmax (DVE+ACT), attention-weighted sum
    via PE directly in transposed layout (bf16), then dense tanh -> 21-row
    decoder -> label mixing -> sentiment head.
"""

import numpy as np
import ml_dtypes

import concourse.bacc as bacc
import concourse.bass as bass
import concourse.tile as tile
from concourse import mybir
from concourse import bass_utils
from concourse.masks import make_identity

BF16 = ml_dtypes.bfloat16

B, S, D, V = 256, 128, 768, 30522
MASK_ID = 103
NCORES = 8
BC = B // NCORES          # 32 rows per core
L = 104                   # global max attention window
C6 = D // 128             # 6 chunks of 128 along D
C12 = 2 * D // 128        # 12 chunks along 2D
QB = 8                    # slots per quarter
NQ = BC // QB             # 4 quarters

LABEL_IDS = [
    [2307, 2204, 3835, 2157, 6581, 2986, 5151, 3893],
    [7929, 24791, 8699, 4257, 16021, 6623],
    [6659, 2919, 11771, 3532, 11325, 4997, 13135],
]
FLAT_IDS = [i for g in LABEL_IDS for i in g]  # 21 ids
NK = len(FLAT_IDS)

PF = 410 + L              # packed f32 constants width
PB = 192 + 6 * NK         # packed bf16 constants width

KERNEL_TRACE = False
LAST_RESULT = None

_cache = {}


def _body(nc, tc, t, lqs):
    f32 = mybir.dt.float32
    bf16 = mybir.dt.bfloat16
    from contextlib import ExitStack

    ctx = ExitStack()
    singles = ctx.enter_context(tc.tile_pool(name="singles", bufs=1))
    work = ctx.enter_context(tc.tile_pool(name="work", bufs=1))
    psum = ctx.enter_context(tc.tile_pool(name="psum", bufs=1, space="PSUM"))

    # ---- resident SBUF tiles, two DMA queues (sync HWDGE + gpsimd SWDGE) -
    xtq_sb, xq_sb, dmq_sb = [], [], []
    for q, lq in enumerate(lqs):
        s = singles.tile([128, C6, QB * lq], f32, tag=f"xtq{q}")
        nc.sync.dma_start(out=s[:], in_=t[f"xtq{q}"][:])
        xtq_sb.append(s)
        s = singles.tile([BC, QB, lq], f32, tag=f"dmq{q}")
        nc.sync.dma_start(out=s[:], in_=t[f"dmq{q}"][:])
        dmq_sb.append(s)
        s = singles.tile([lq, QB, D], bf16, tag=f"xq{q}")
        nc.gpsimd.dma_start(out=s[:], in_=t[f"xq{q}"][:])
        xq_sb.append(s)
    packf_sb = singles.tile([128, PF], f32, tag="packf")
    nc.sync.dma_start(out=packf_sb[:], in_=t["packf"][:])
    packb_sb = singles.tile([128, PB], bf16, tag="packb")
    nc.sync.dma_start(out=packb_sb[:], in_=t["packb"][:])
    wdt_sb = singles.tile([128, C12, D], bf16, tag="wdt")
    nc.gpsimd.dma_start(out=wdt_sb[:], in_=t["wdt"][:])

    mt_sb = packf_sb[:, 0:192].rearrange("p (c b) -> p c b", c=C6)
    poolert_sb = packf_sb[:, 192:384].rearrange("p (c b) -> p c b", c=C6)
    sentit_sb = packf_sb[:, 384:396].rearrange("p (c b) -> p c b", c=C6)
    db_sb = packf_sb[:, 396:402]
    wf_sb = packf_sb[0:NK, 402:408]
    dbdec_sb = packf_sb[0:NK, 408:409]
    sb_sb = packf_sb[0:2, 409:410]
    amask_sb = packf_sb[0:BC, 410 : 410 + L]
    mtb_sb = packb_sb[:, 0:192].rearrange("p (c b) -> p c b", c=C6)
    dect_sb = packb_sb[:, 192 : 192 + C6 * NK].rearrange("p (c k) -> p c k", c=C6)

    ident_f = singles.tile([BC, BC], f32, tag="identf")
    make_identity(nc, ident_f[:])

    ps_attt = psum.tile([128, C6, BC], f32, tag="attt")

    # ---- per-quarter attention pipeline ----------------------------------
    for q, lq in enumerate(lqs):
        qn = QB * lq
        # scores all-pairs: S[b', (i,s)] = m_b' . X_{slot i}[s]
        ps_q = psum.tile([BC, 2, 512], f32, tag="big", bufs=2)
        nslices = [(j * 512, min(512, qn - j * 512)) for j in range((qn + 511) // 512)]
        for (j0, jn) in nslices:
            for c in range(C6):
                nc.tensor.matmul(
                    ps_q[:, j0 // 512, 0:jn],
                    mt_sb[:, c, :],
                    xtq_sb[q][:, c, j0 : j0 + jn],
                    start=(c == 0),
                    stop=(c == C6 - 1),
                )
        # diagonal extraction: zero off-diagonal blocks, reduce over i
        masked = work.tile([BC, QB, lq], f32, tag="masked", bufs=2)
        nc.vector.tensor_tensor(
            out=masked[:],
            in0=ps_q[:].rearrange("p h n -> p (h n)")[:, 0:qn].rearrange(
                "p (i s) -> p i s", i=QB
            ),
            in1=dmq_sb[q][:],
            op=mybir.AluOpType.mult,
        )
        red = work.tile([BC, L], f32, tag="red", bufs=2)
        nc.vector.memset(red[:], 0.0)
        nc.vector.tensor_reduce(
            out=red[:, 0:lq],
            in_=masked[:].rearrange("p i s -> p s i"),
            axis=mybir.AxisListType.X,
            op=mybir.AluOpType.add,
        )
        # masked softmax over the free dim (rows outside this quarter are
        # garbage and never read downstream)
        scores_sb = work.tile([BC, L], f32, tag="scores_sb", bufs=2)
        nc.vector.tensor_tensor(
            out=scores_sb[:], in0=red[:], in1=amask_sb[:], op=mybir.AluOpType.add
        )
        mx = work.tile([BC, 1], f32, tag="mx", bufs=2)
        nc.vector.tensor_reduce(
            out=mx[:],
            in_=scores_sb[:],
            axis=mybir.AxisListType.X,
            op=mybir.AluOpType.max,
        )
        negmax = work.tile([BC, 1], f32, tag="negmax", bufs=2)
        nc.vector.tensor_scalar_mul(negmax[:], mx[:], -1.0)
        p_sb = work.tile([BC, L], f32, tag="p_sb", bufs=2)
        sumexp = work.tile([BC, 1], f32, tag="sumexp", bufs=2)
        nc.scalar.activation(
            out=p_sb[:],
            in_=scores_sb[:],
            func=mybir.ActivationFunctionType.Exp,
            bias=negmax[:],
            scale=1.0,
            accum_out=sumexp[:],
        )
        rsum = work.tile([BC, 1], f32, tag="rsum", bufs=2)
        nc.vector.reciprocal(out=rsum[:], in_=sumexp[:])
        nc.vector.tensor_scalar_mul(p_sb[:], p_sb[:], rsum[:])

        # p^T for this quarter (cast bf16)
        ps_pt = psum.tile([L, BC], f32, tag="ptq", bufs=2)
        nc.tensor.transpose(ps_pt[:], p_sb[:], ident_f[:])
        pt_sb = work.tile([L, BC], bf16, tag="pt_sb", bufs=2)
        nc.vector.tensor_copy(pt_sb[:], ps_pt[:])

        # att^T[d, b] for the 8 slots of this quarter
        for i in range(QB):
            b = q * QB + i
            for c in range(C6):
                nc.tensor.matmul(
                    ps_attt[:, c, b : b + 1],
                    xq_sb[q][:, i, c * 128 : (c + 1) * 128],  # lhsT [lq, 128]
                    pt_sb[0:lq, b : b + 1],                   # rhs  [lq, 1]
                    start=True,
                    stop=True,
                )

    attt_sb = work.tile([128, C6, BC], bf16, tag="attt_sb")
    nc.vector.tensor_copy(attt_sb[:], ps_attt[:])

    # ---- dense: h^T[o, b] = tanh(sum_i W[o,i] feats[b,i] + db[o]) --------
    ps_ht = psum.tile([128, C6, BC], f32, tag="ht")
    for c in range(C6):
        for k in range(C12):
            rhs = attt_sb[:, k, :] if k < C6 else mtb_sb[:, k - C6, :]
            nc.tensor.matmul(
                ps_ht[:, c, :],
                wdt_sb[:, k, c * 128 : (c + 1) * 128],
                rhs,
                start=(k == 0),
                stop=(k == C12 - 1),
            )
    ht_sb = work.tile([128, C6, BC], bf16, tag="ht_sb")
    for c in range(C6):
        nc.scalar.activation(
            out=ht_sb[:, c, :],
            in_=ps_ht[:, c, :],
            func=mybir.ActivationFunctionType.Tanh,
            bias=db_sb[:, c : c + 1],
            scale=1.0,
        )

    # ---- decoder (21 label rows) -> label mixing -------------------------
    ps_p21 = psum.tile([NK, BC], f32, tag="ptq", bufs=2)
    for c in range(C6):
        nc.tensor.matmul(
            ps_p21[:],
            dect_sb[:, c, :],
            ht_sb[:, c, :],
            start=(c == 0),
            stop=(c == C6 - 1),
        )
    p21t_sb = work.tile([NK, BC], f32, tag="p21t_sb")
    nc.scalar.activation(
        out=p21t_sb[:],
        in_=ps_p21[:],
        func=mybir.ActivationFunctionType.Tanh,
        bias=dbdec_sb[:],
        scale=1.0,
    )
    ps_out6 = psum.tile([BC, 6], f32, tag="ptq", bufs=2)
    nc.tensor.matmul(ps_out6[:], p21t_sb[:], wf_sb[:], start=True, stop=True)
    out6_sb = work.tile([BC, 6], f32, tag="out6_sb")
    nc.vector.tensor_copy(out6_sb[:], ps_out6[:])
    nc.sync.dma_start(out=t["out6"][:], in_=out6_sb[:])

    # ---- sentiment head --------------------------------------------------
    ps_cat = psum.tile([2, BC], f32, tag="ptq", bufs=2)
    for c in range(C6):
        nc.tensor.matmul(
            ps_cat[:],
            sentit_sb[:, c, :],
            poolert_sb[:, c, :],
            start=(c == 0),
            stop=(c == C6 - 1),
        )
    catt_sb = work.tile([2, BC], f32, tag="catt_sb")
    nc.vector.tensor_scalar_add(catt_sb[:], ps_cat[:], sb_sb[:])
    nc.sync.dma_start(out=t["catt"][:], in_=catt_sb[:])

    ctx.close()


def _build(lqs):
    key = tuple(lqs)
    if key in _cache:
        return _cache[key]
    f32 = mybir.dt.float32
    bf16 = mybir.dt.bfloat16
    nc = bacc.Bacc("TRN2", target_bir_lowering=False, debug=False)
    t = {}

    def din(name, shape, dt):
        t[name] = nc.dram_tensor(name, shape, dt, kind="ExternalInput").ap()

    def dout(name, shape, dt):
        t[name] = nc.dram_tensor(name, shape, dt, kind="ExternalOutput").ap()

    for q, lq in enumerate(lqs):
        din(f"xtq{q}", [128, C6, QB * lq], f32)
        din(f"xq{q}", [lq, QB, D], bf16)
        din(f"dmq{q}", [BC, QB, lq], f32)
    din("wdt", [128, C12, D], bf16)
    din("packf", [128, PF], f32)
    din("packb", [128, PB], bf16)
    dout("out6", [BC, 6], f32)
    dout("catt", [2, BC], f32)

    with tile.TileContext(nc) as tc:
        _body(nc, tc, t, lqs)
    nc.compile()
    _cache[key] = (nc, t)
    return _cache[key]


def _chunkT(a2d):
    """[N, D-like] -> [128, D//128, N] chunked transpose layout."""
    d = a2d.shape[1]
    return np.ascontiguousarray(
        a2d.T.reshape(d // 128, 128, a2d.shape[0]).transpose(1, 0, 2)
    )


def kernel(**inputs):
    global LAST_RESULT
    bert = np.asarray(inputs["bert_out"], dtype=np.float32)      # [B, S, D]
    ids = np.asarray(inputs["input_ids"])
    length = np.asarray(inputs["length"]).astype(np.int64)
    senti_w = np.asarray(inputs["senti_w"], dtype=np.float32)
    senti_b = np.asarray(inputs["senti_b"], dtype=np.float32)
    dense_w = np.asarray(inputs["dense_w"], dtype=np.float32)    # [D, 2D]
    dense_b = np.asarray(inputs["dense_b"], dtype=np.float32)
    dec_w = np.asarray(inputs["dec_w"], dtype=np.float32)        # [V, D]
    dec_b = np.asarray(inputs["dec_b"], dtype=np.float32)
    w0 = np.asarray(inputs["w0"], dtype=np.float32)
    w1 = np.asarray(inputs["w1"], dtype=np.float32)
    w2 = np.asarray(inputs["w2"], dtype=np.float32)

    mask_pos = np.argmax(ids == MASK_ID, axis=1)                 # [B]
    m = bert[np.arange(B), mask_pos]                             # [B, D]
    pooler = bert[:, 0]                                          # [B, D]

    # global sort by length; rank r -> core r%8, slot r//8
    order = np.argsort(length, kind="stable")
    lens = length[order]
    lqs = []
    for q in range(NQ):
        lq = int(lens[(q + 1) * QB * NCORES - 1])
        lqs.append(min(L, ((lq + 7) // 8) * 8))

    # shared weight layouts
    wdt = np.ascontiguousarray(
        dense_w.T.reshape(C12, 128, D).transpose(1, 0, 2)
    ).astype(BF16)
    dec21 = dec_w[FLAT_IDS]
    dect = _chunkT(dec21).astype(BF16)
    sentit = _chunkT(senti_w)
    wf = np.zeros((NK, 6), np.float32)
    off = 0
    for g, wg in enumerate((w0, w1, w2)):
        k = wg.shape[1]
        for j in range(2):
            wf[off : off + k, j * 3 + g] = wg[j]
        off += k
    db = np.ascontiguousarray(dense_b.reshape(C6, 128).T)
    dbdec = dec_b[FLAT_IDS][:, None].astype(np.float32)
    sb = senti_b[:, None]

    nc, t = _build(lqs)

    in_maps = []
    core_rows = []
    for k in range(NCORES):
        rows = order[np.arange(BC) * NCORES + k]                 # 32 batch idx
        core_rows.append(rows)
        rlen = length[rows]
        im = {}
        for q, lq in enumerate(lqs):
            r = rows[q * QB : (q + 1) * QB]
            rl = rlen[q * QB : (q + 1) * QB]
            padded = np.zeros((QB, lq, D), np.float32)
            for i in range(QB):
                padded[i, : rl[i]] = bert[r[i], 3 : 3 + rl[i]]
            im[f"xq{q}"] = np.ascontiguousarray(
                padded.transpose(1, 0, 2)
            ).astype(BF16)
            im[f"xtq{q}"] = np.ascontiguousarray(
                padded.transpose(2, 0, 1)
                .reshape(C6, 128, QB, lq)
                .transpose(1, 0, 2, 3)
            ).reshape(128, C6, QB * lq)
            dmq = np.zeros((BC, QB, lq), np.float32)
            for i in range(QB):
                dmq[q * QB + i, i, :] = 1.0
            im[f"dmq{q}"] = dmq
        mtk = _chunkT(m[rows])
        amask = np.where(
            np.arange(L)[None, :] < rlen[:, None], 0.0, -1e30
        ).astype(np.float32)
        packf = np.zeros((128, PF), np.float32)
        packf[:, 0:192] = mtk.reshape(128, 192)
        packf[:, 192:384] = _chunkT(pooler[rows]).reshape(128, 192)
        packf[:, 384:396] = sentit.reshape(128, 12)
        packf[:, 396:402] = db
        packf[0:NK, 402:408] = wf
        packf[0:NK, 408:409] = dbdec
        packf[0:2, 409:410] = sb
        packf[0:BC, 410 : 410 + L] = amask
        packb = np.zeros((128, PB), BF16)
        packb[:, 0:192] = mtk.astype(BF16).reshape(128, 192)
        packb[:, 192:PB] = dect.reshape(128, C6 * NK)
        im["wdt"] = wdt
        im["packf"] = packf
        im["packb"] = packb
        in_maps.append(im)

    res = bass_utils.run_bass_kernel_spmd(
        nc, in_maps, core_ids=list(range(NCORES)), trace=KERNEL_TRACE
    )
    LAST_RESULT = res

    category_out = np.empty((B, 2), np.float32)
    out = np.empty((B, 2, 3), np.float32)
    for k in range(NCORES):
        rows = core_rows[k]
        category_out[rows] = res.results[k]["catt"].T
        out[rows] = res.results[k]["out6"].reshape(BC, 2, 3)
    return category_out, out


# revision 36
# speedup vs baseline: 2.6961x; 1.9214x over previous
"""Trainium2 Bass kernel for nn_ClassModel_72318659330833.

Strategy:
  - Data-parallel over batch: 8 cores x 32 rows; the batch is globally
    sorted by attention length and dealt round-robin so all cores share an
    identical slot-length profile (SPMD: one program).
  - Slots are grouped into 4 quarters of 8; each quarter is padded to its
    own max length Lq (multiple of 8). This makes the ragged attention
    window static per quarter while skipping ~35% of bytes and FLOPs.
  - Host-side prep (index/layout only): mask-row gather, transposed/chunked
    weight layouts, additive masks, decoder sliced to the 21 label rows.
  - Device per core, pipelined over quarters: attention scores via PE
    all-pairs (f32 X^T, exact), diagonal extraction via mask-multiply +
    strided reduce (DVE), masked softmax (DVE+ACT), attention-weighted sum
    via PE directly in transposed layout (bf16), then dense tanh -> 21-row
    decoder -> label mixing -> sentiment head.
"""

import numpy as np
import ml_dtypes

import concourse.bacc as bacc
import concourse.bass as bass
import concourse.tile as tile
from concourse import mybir
from concourse import bass_utils
from concourse.masks import make_identity

BF16 = ml_dtypes.bfloat16

B, S, D, V = 256, 128, 768, 30522
MASK_ID = 103
NCORES = 8
BC = B // NCORES          # 32 rows per core
L = 104                   # global max attention window
C6 = D // 128             # 6 chunks of 128 along D
C12 = 2 * D // 128        # 12 chunks along 2D
QB = 8                    # slots per quarter
NQ = BC // QB             # 4 quarters

LABEL_IDS = [
    [2307, 2204, 3835, 2157, 6581, 2986, 5151, 3893],
    [7929, 24791, 8699, 4257, 16021, 6623],
    [6659, 2919, 11771, 3532, 11325, 4997, 13135],
]
FLAT_IDS = [i for g in LABEL_IDS for i in g]  # 21 ids
NK = len(FLAT_IDS)

PF = 410 + L              # packed f32 constants width
PB = 192 + 6 * NK         # packed bf16 constants width

KERNEL_TRACE = False
LAST_RESULT = None

_cache = {}


def _body(nc, tc, t, lqs):
    f32 = mybir.dt.float32
    bf16 = mybir.dt.bfloat16
    from contextlib import ExitStack

    ctx = ExitStack()
    singles = ctx.enter_context(tc.tile_pool(name="singles", bufs=1))
    work = ctx.enter_context(tc.tile_pool(name="work", bufs=1))
    psum = ctx.enter_context(tc.tile_pool(name="psum", bufs=1, space="PSUM"))

    # ---- resident SBUF tiles, two DMA queues (sync HWDGE + gpsimd SWDGE) -
    f16 = mybir.dt.float16
    xtq_sb, xq_sb, dmq_sb = [], [], []
    mt16_sb = singles.tile([128, C6, BC], f16, tag="mt16")
    nc.sync.dma_start(out=mt16_sb[:], in_=t["mt16"][:])
    for q, lq in enumerate(lqs):
        s = singles.tile([128, C6, QB * lq], f16, tag=f"xtq{q}")
        nc.sync.dma_start(out=s[:], in_=t[f"xtq{q}"][:])
        xtq_sb.append(s)
        s = singles.tile([BC, QB, lq], f32, tag=f"dmq{q}")
        nc.sync.dma_start(out=s[:], in_=t[f"dmq{q}"][:])
        dmq_sb.append(s)
        s = singles.tile([lq, QB, D], bf16, tag=f"xq{q}")
        nc.gpsimd.dma_start(out=s[:], in_=t[f"xq{q}"][:])
        xq_sb.append(s)
    packf_sb = singles.tile([128, PF], f32, tag="packf")
    nc.sync.dma_start(out=packf_sb[:], in_=t["packf"][:])
    packb_sb = singles.tile([128, PB], bf16, tag="packb")
    nc.sync.dma_start(out=packb_sb[:], in_=t["packb"][:])
    wdt_sb = singles.tile([128, C12, D], bf16, tag="wdt")
    nc.sync.dma_start(out=wdt_sb[:], in_=t["wdt"][:])

    mt_sb = mt16_sb
    poolert_sb = packf_sb[:, 192:384].rearrange("p (c b) -> p c b", c=C6)
    sentit_sb = packf_sb[:, 384:396].rearrange("p (c b) -> p c b", c=C6)
    db_sb = packf_sb[:, 396:402]
    wf_sb = packf_sb[0:NK, 402:408]
    dbdec_sb = packf_sb[0:NK, 408:409]
    sb_sb = packf_sb[0:2, 409:410]
    amask_sb = packf_sb[0:BC, 410 : 410 + L]
    mtb_sb = packb_sb[:, 0:192].rearrange("p (c b) -> p c b", c=C6)
    dect_sb = packb_sb[:, 192 : 192 + C6 * NK].rearrange("p (c k) -> p c k", c=C6)

    ident_f = singles.tile([BC, BC], f32, tag="identf")
    make_identity(nc, ident_f[:])

    ps_attt = psum.tile([128, C6, BC], f32, tag="attt")

    # ---- per-quarter attention pipeline ----------------------------------
    for q, lq in enumerate(lqs):
        qn = QB * lq
        # scores all-pairs: S[b', (i,s)] = m_b' . X_{slot i}[s]
        ps_q = psum.tile([BC, 2, 512], f32, tag="big", bufs=2)
        nslices = [(j * 512, min(512, qn - j * 512)) for j in range((qn + 511) // 512)]
        for (j0, jn) in nslices:
            for c in range(C6):
                nc.tensor.matmul(
                    ps_q[:, j0 // 512, 0:jn],
                    mt_sb[:, c, :],
                    xtq_sb[q][:, c, j0 : j0 + jn],
                    start=(c == 0),
                    stop=(c == C6 - 1),
                )
        # diagonal extraction: zero off-diagonal blocks, reduce over i
        masked = work.tile([BC, QB, lq], f32, tag="masked", bufs=2)
        nc.vector.tensor_tensor(
            out=masked[:],
            in0=ps_q[:].rearrange("p h n -> p (h n)")[:, 0:qn].rearrange(
                "p (i s) -> p i s", i=QB
            ),
            in1=dmq_sb[q][:],
            op=mybir.AluOpType.mult,
        )
        red = work.tile([BC, L], f32, tag="red", bufs=2)
        nc.vector.memset(red[:], 0.0)
        nc.vector.tensor_reduce(
            out=red[:, 0:lq],
            in_=masked[:].rearrange("p i s -> p s i"),
            axis=mybir.AxisListType.X,
            op=mybir.AluOpType.add,
        )
        # masked softmax over the free dim (rows outside this quarter are
        # garbage and never read downstream)
        scores_sb = work.tile([BC, L], f32, tag="scores_sb", bufs=2)
        nc.vector.tensor_tensor(
            out=scores_sb[:], in0=red[:], in1=amask_sb[:], op=mybir.AluOpType.add
        )
        mx = work.tile([BC, 1], f32, tag="mx", bufs=2)
        nc.vector.tensor_reduce(
            out=mx[:],
            in_=scores_sb[:],
            axis=mybir.AxisListType.X,
            op=mybir.AluOpType.max,
        )
        negmax = work.tile([BC, 1], f32, tag="negmax", bufs=2)
        nc.vector.tensor_scalar_mul(negmax[:], mx[:], -1.0)
        p_sb = work.tile([BC, L], f32, tag="p_sb", bufs=2)
        sumexp = work.tile([BC, 1], f32, tag="sumexp", bufs=2)
        nc.scalar.activation(
            out=p_sb[:],
            in_=scores_sb[:],
            func=mybir.ActivationFunctionType.Exp,
            bias=negmax[:],
            scale=1.0,
            accum_out=sumexp[:],
        )
        rsum = work.tile([BC, 1], f32, tag="rsum", bufs=2)
        nc.vector.reciprocal(out=rsum[:], in_=sumexp[:])
        nc.vector.tensor_scalar_mul(p_sb[:], p_sb[:], rsum[:])

        # p^T for this quarter (cast bf16)
        ps_pt = psum.tile([L, BC], f32, tag="ptq", bufs=2)
        nc.tensor.transpose(ps_pt[:], p_sb[:], ident_f[:])
        pt_sb = work.tile([L, BC], bf16, tag="pt_sb", bufs=2)
        nc.vector.tensor_copy(pt_sb[:], ps_pt[:])

        # att^T[d, b] for the 8 slots of this quarter
        for i in range(QB):
            b = q * QB + i
            for c in range(C6):
                nc.tensor.matmul(
                    ps_attt[:, c, b : b + 1],
                    xq_sb[q][:, i, c * 128 : (c + 1) * 128],  # lhsT [lq, 128]
                    pt_sb[0:lq, b : b + 1],                   # rhs  [lq, 1]
                    start=True,
                    stop=True,
                )

    attt_sb = work.tile([128, C6, BC], bf16, tag="attt_sb")
    nc.vector.tensor_copy(attt_sb[:], ps_attt[:])

    # ---- dense: h^T[o, b] = tanh(sum_i W[o,i] feats[b,i] + db[o]) --------
    ps_ht = psum.tile([128, C6, BC], f32, tag="ht")
    for c in range(C6):
        for k in range(C12):
            rhs = attt_sb[:, k, :] if k < C6 else mtb_sb[:, k - C6, :]
            nc.tensor.matmul(
                ps_ht[:, c, :],
                wdt_sb[:, k, c * 128 : (c + 1) * 128],
                rhs,
                start=(k == 0),
                stop=(k == C12 - 1),
            )
    ht_sb = work.tile([128, C6, BC], bf16, tag="ht_sb")
    for c in range(C6):
        nc.scalar.activation(
            out=ht_sb[:, c, :],
            in_=ps_ht[:, c, :],
            func=mybir.ActivationFunctionType.Tanh,
            bias=db_sb[:, c : c + 1],
            scale=1.0,
        )

    # ---- decoder (21 label rows) -> label mixing -------------------------
    ps_p21 = psum.tile([NK, BC], f32, tag="ptq", bufs=2)
    for c in range(C6):
        nc.tensor.matmul(
            ps_p21[:],
            dect_sb[:, c, :],
            ht_sb[:, c, :],
            start=(c == 0),
            stop=(c == C6 - 1),
        )
    p21t_sb = work.tile([NK, BC], f32, tag="p21t_sb")
    nc.scalar.activation(
        out=p21t_sb[:],
        in_=ps_p21[:],
        func=mybir.ActivationFunctionType.Tanh,
        bias=dbdec_sb[:],
        scale=1.0,
    )
    ps_out6 = psum.tile([BC, 6], f32, tag="ptq", bufs=2)
    nc.tensor.matmul(ps_out6[:], p21t_sb[:], wf_sb[:], start=True, stop=True)
    out6_sb = work.tile([BC, 6], f32, tag="out6_sb")
    nc.vector.tensor_copy(out6_sb[:], ps_out6[:])
    nc.sync.dma_start(out=t["out6"][:], in_=out6_sb[:])

    # ---- sentiment head --------------------------------------------------
    ps_cat = psum.tile([2, BC], f32, tag="ptq", bufs=2)
    for c in range(C6):
        nc.tensor.matmul(
            ps_cat[:],
            sentit_sb[:, c, :],
            poolert_sb[:, c, :],
            start=(c == 0),
            stop=(c == C6 - 1),
        )
    catt_sb = work.tile([2, BC], f32, tag="catt_sb")
    nc.vector.tensor_scalar_add(catt_sb[:], ps_cat[:], sb_sb[:])
    nc.sync.dma_start(out=t["catt"][:], in_=catt_sb[:])

    ctx.close()


def _build(lqs):
    key = tuple(lqs)
    if key in _cache:
        return _cache[key]
    f32 = mybir.dt.float32
    bf16 = mybir.dt.bfloat16
    nc = bacc.Bacc("TRN2", target_bir_lowering=False, debug=False)
    t = {}

    def din(name, shape, dt):
        t[name] = nc.dram_tensor(name, shape, dt, kind="ExternalInput").ap()

    def dout(name, shape, dt):
        t[name] = nc.dram_tensor(name, shape, dt, kind="ExternalOutput").ap()

    f16 = mybir.dt.float16
    din("mt16", [128, C6, BC], f16)
    for q, lq in enumerate(lqs):
        din(f"xtq{q}", [128, C6, QB * lq], f16)
        din(f"xq{q}", [lq, QB, D], bf16)
        din(f"dmq{q}", [BC, QB, lq], f32)
    din("wdt", [128, C12, D], bf16)
    din("packf", [128, PF], f32)
    din("packb", [128, PB], bf16)
    dout("out6", [BC, 6], f32)
    dout("catt", [2, BC], f32)

    with tile.TileContext(nc) as tc:
        _body(nc, tc, t, lqs)
    nc.compile()
    _cache[key] = (nc, t)
    return _cache[key]


def _chunkT(a2d):
    """[N, D-like] -> [128, D//128, N] chunked transpose layout."""
    d = a2d.shape[1]
    return np.ascontiguousarray(
        a2d.T.reshape(d // 128, 128, a2d.shape[0]).transpose(1, 0, 2)
    )


def kernel(**inputs):
    global LAST_RESULT
    bert = np.asarray(inputs["bert_out"], dtype=np.float32)      # [B, S, D]
    ids = np.asarray(inputs["input_ids"])
    length = np.asarray(inputs["length"]).astype(np.int64)
    senti_w = np.asarray(inputs["senti_w"], dtype=np.float32)
    senti_b = np.asarray(inputs["senti_b"], dtype=np.float32)
    dense_w = np.asarray(inputs["dense_w"], dtype=np.float32)    # [D, 2D]
    dense_b = np.asarray(inputs["dense_b"], dtype=np.float32)
    dec_w = np.asarray(inputs["dec_w"], dtype=np.float32)        # [V, D]
    dec_b = np.asarray(inputs["dec_b"], dtype=np.float32)
    w0 = np.asarray(inputs["w0"], dtype=np.float32)
    w1 = np.asarray(inputs["w1"], dtype=np.float32)
    w2 = np.asarray(inputs["w2"], dtype=np.float32)

    mask_pos = np.argmax(ids == MASK_ID, axis=1)                 # [B]
    m = bert[np.arange(B), mask_pos]                             # [B, D]
    pooler = bert[:, 0]                                          # [B, D]

    # global sort by length; rank r -> core r%8, slot r//8
    order = np.argsort(length, kind="stable")
    lens = length[order]
    lqs = []
    for q in range(NQ):
        lq = int(lens[(q + 1) * QB * NCORES - 1])
        lqs.append(min(L, ((lq + 7) // 8) * 8))

    # shared weight layouts
    wdt = np.ascontiguousarray(
        dense_w.T.reshape(C12, 128, D).transpose(1, 0, 2)
    ).astype(BF16)
    dec21 = dec_w[FLAT_IDS]
    dect = _chunkT(dec21).astype(BF16)
    sentit = _chunkT(senti_w)
    wf = np.zeros((NK, 6), np.float32)
    off = 0
    for g, wg in enumerate((w0, w1, w2)):
        k = wg.shape[1]
        for j in range(2):
            wf[off : off + k, j * 3 + g] = wg[j]
        off += k
    db = np.ascontiguousarray(dense_b.reshape(C6, 128).T)
    dbdec = dec_b[FLAT_IDS][:, None].astype(np.float32)
    sb = senti_b[:, None]

    nc, t = _build(lqs)

    in_maps = []
    core_rows = []
    for k in range(NCORES):
        rows = order[np.arange(BC) * NCORES + k]                 # 32 batch idx
        core_rows.append(rows)
        rlen = length[rows]
        im = {}
        for q, lq in enumerate(lqs):
            r = rows[q * QB : (q + 1) * QB]
            rl = rlen[q * QB : (q + 1) * QB]
            padded = np.zeros((QB, lq, D), np.float32)
            for i in range(QB):
                padded[i, : rl[i]] = bert[r[i], 3 : 3 + rl[i]]
            im[f"xq{q}"] = np.ascontiguousarray(
                padded.transpose(1, 0, 2)
            ).astype(BF16)
            im[f"xtq{q}"] = (
                np.ascontiguousarray(
                    padded.transpose(2, 0, 1)
                    .reshape(C6, 128, QB, lq)
                    .transpose(1, 0, 2, 3)
                )
                .reshape(128, C6, QB * lq)
                .astype(np.float16)
            )
            dmq = np.zeros((BC, QB, lq), np.float32)
            for i in range(QB):
                dmq[q * QB + i, i, :] = 1.0
            im[f"dmq{q}"] = dmq
        mtk = _chunkT(m[rows])
        amask = np.where(
            np.arange(L)[None, :] < rlen[:, None], 0.0, -1e30
        ).astype(np.float32)
        packf = np.zeros((128, PF), np.float32)
        packf[:, 0:192] = mtk.reshape(128, 192)
        packf[:, 192:384] = _chunkT(pooler[rows]).reshape(128, 192)
        packf[:, 384:396] = sentit.reshape(128, 12)
        packf[:, 396:402] = db
        packf[0:NK, 402:408] = wf
        packf[0:NK, 408:409] = dbdec
        packf[0:2, 409:410] = sb
        packf[0:BC, 410 : 410 + L] = amask
        packb = np.zeros((128, PB), BF16)
        packb[:, 0:192] = mtk.astype(BF16).reshape(128, 192)
        packb[:, 192:PB] = dect.reshape(128, C6 * NK)
        im["wdt"] = wdt
        im["packf"] = packf
        im["packb"] = packb
        im["mt16"] = mtk.astype(np.float16)
        in_maps.append(im)

    res = bass_utils.run_bass_kernel_spmd(
        nc, in_maps, core_ids=list(range(NCORES)), trace=KERNEL_TRACE
    )
    LAST_RESULT = res

    category_out = np.empty((B, 2), np.float32)
    out = np.empty((B, 2, 3), np.float32)
    for k in range(NCORES):
        rows = core_rows[k]
        category_out[rows] = res.results[k]["catt"].T
        out[rows] = res.results[k]["out6"].reshape(BC, 2, 3)
    return category_out, out


# revision 47
# speedup vs baseline: 3.1735x; 1.1771x over previous
"""Trainium2 Bass kernel for nn_ClassModel_72318659330833.

Strategy:
  - Data-parallel over batch: 8 cores x 32 rows; the batch is globally
    sorted by attention length and dealt round-robin so all cores share an
    identical slot-length profile (SPMD: one program).
  - Slots are grouped into 4 quarters of 8; each quarter is padded to its
    own max length Lq (multiple of 8). This makes the ragged attention
    window static per quarter while skipping ~35% of bytes and FLOPs.
  - Host-side prep (index/layout only): mask-row gather, transposed/chunked
    weight layouts, additive masks, decoder sliced to the 21 label rows.
  - Device per core, pipelined over quarters: attention scores via PE
    all-pairs (f32 X^T, exact), diagonal extraction via mask-multiply +
    strided reduce (DVE), masked softmax (DVE+ACT), attention-weighted sum
    via PE directly in transposed layout (bf16), then dense tanh -> 21-row
    decoder -> label mixing -> sentiment head.
"""

import numpy as np
import ml_dtypes

import concourse.bacc as bacc
import concourse.bass as bass
import concourse.tile as tile
from concourse import mybir
from concourse import bass_utils
from concourse.masks import make_identity

BF16 = ml_dtypes.bfloat16

B, S, D, V = 256, 128, 768, 30522
MASK_ID = 103
NCORES = 8
BC = B // NCORES          # 32 rows per core
L = 104                   # global max attention window
C6 = D // 128             # 6 chunks of 128 along D
C12 = 2 * D // 128        # 12 chunks along 2D
QB = 8                    # slots per quarter
NQ = BC // QB             # 4 quarters

LABEL_IDS = [
    [2307, 2204, 3835, 2157, 6581, 2986, 5151, 3893],
    [7929, 24791, 8699, 4257, 16021, 6623],
    [6659, 2919, 11771, 3532, 11325, 4997, 13135],
]
FLAT_IDS = [i for g in LABEL_IDS for i in g]  # 21 ids
NK = len(FLAT_IDS)

PF = 410 + L              # packed f32 constants width
PB = 192 + 6 * NK         # packed bf16 constants width

KERNEL_TRACE = False
LAST_RESULT = None

_cache = {}


def _body(nc, tc, t, lqs):
    f32 = mybir.dt.float32
    bf16 = mybir.dt.bfloat16
    from contextlib import ExitStack

    ctx = ExitStack()
    singles = ctx.enter_context(tc.tile_pool(name="singles", bufs=1))
    work = ctx.enter_context(tc.tile_pool(name="work", bufs=1))
    psum = ctx.enter_context(tc.tile_pool(name="psum", bufs=1, space="PSUM"))

    # ---- resident SBUF tiles, two DMA queues (sync HWDGE + gpsimd SWDGE) -
    f16 = mybir.dt.float16
    # one merged f16 DMA: [mt16 | xtq0..3]; one merged f32 mask DMA; one
    # merged bf16 weights DMA; one packed f32 constants DMA.
    nxt = C6 * BC + sum(C6 * QB * lq for lq in lqs)
    nxt01 = C6 * BC + sum(C6 * QB * lq for lq in lqs[:2])
    xtall_sb = singles.tile([128, nxt], f16, tag="xtall")
    nc.sync.dma_start(out=xtall_sb[:, 0:nxt01], in_=t["xtall"][:, 0:nxt01])
    ndm = sum(QB * lq for lq in lqs)
    dmall_sb = singles.tile([BC, ndm], f32, tag="dmall")
    nc.sync.dma_start(out=dmall_sb[:], in_=t["dmall"][:])
    packf_sb = singles.tile([128, PF], f32, tag="packf")
    nc.sync.dma_start(out=packf_sb[:], in_=t["packf"][:])
    nc.sync.dma_start(out=xtall_sb[:, nxt01:nxt], in_=t["xtall"][:, nxt01:nxt])
    wpack_sb = singles.tile([128, C12 * D + PB], bf16, tag="wpack")
    nc.sync.dma_start(out=wpack_sb[:], in_=t["wpack"][:])
    xq_sb = []
    for q, lq in enumerate(lqs):
        s = singles.tile([lq, QB, D], bf16, tag=f"xq{q}")
        nc.gpsimd.dma_start(out=s[:], in_=t[f"xq{q}"][:])
        xq_sb.append(s)

    mt_sb = xtall_sb[:, 0 : C6 * BC].rearrange("p (c b) -> p c b", c=C6)
    xtq_sb, dmq_sb = [], []
    xoff, doff = C6 * BC, 0
    for q, lq in enumerate(lqs):
        xtq_sb.append(
            xtall_sb[:, xoff : xoff + C6 * QB * lq].rearrange(
                "p (c n) -> p c n", c=C6
            )
        )
        xoff += C6 * QB * lq
        dmq_sb.append(
            dmall_sb[:, doff : doff + QB * lq].rearrange("p (i s) -> p i s", i=QB)
        )
        doff += QB * lq
    wdt_sb = wpack_sb[:, 0 : C12 * D].rearrange("p (k d) -> p k d", k=C12)
    packb_sb = wpack_sb[:, C12 * D : C12 * D + PB]
    poolert_sb = packf_sb[:, 192:384].rearrange("p (c b) -> p c b", c=C6)
    sentit_sb = packf_sb[:, 384:396].rearrange("p (c b) -> p c b", c=C6)
    db_sb = packf_sb[:, 396:402]
    wf_sb = packf_sb[0:NK, 402:408]
    dbdec_sb = packf_sb[0:NK, 408:409]
    sb_sb = packf_sb[0:2, 409:410]
    amask_sb = packf_sb[0:BC, 410 : 410 + L]
    mtb_sb = packb_sb[:, 0:192].rearrange("p (c b) -> p c b", c=C6)
    dect_sb = packb_sb[:, 192 : 192 + C6 * NK].rearrange("p (c k) -> p c k", c=C6)

    ident_f = singles.tile([BC, BC], f32, tag="identf")
    make_identity(nc, ident_f[:])

    # ---- sentiment head (independent of attention; fills the pipe head) --
    ps_cat = psum.tile([2, BC], f32, tag="ptq", bufs=2)
    for c in range(C6):
        nc.tensor.matmul(
            ps_cat[:],
            sentit_sb[:, c, :],
            poolert_sb[:, c, :],
            start=(c == 0),
            stop=(c == C6 - 1),
        )
    catt_sb = work.tile([2, BC], f32, tag="catt_sb")
    nc.vector.tensor_scalar_add(catt_sb[:], ps_cat[:], sb_sb[:])
    nc.sync.dma_start(out=t["catt"][:], in_=catt_sb[:])

    ps_attt = psum.tile([128, C6, BC], f32, tag="attt")

    # ---- per-quarter attention pipeline ----------------------------------
    for q, lq in enumerate(lqs):
        qn = QB * lq
        # scores all-pairs: S[b', (i,s)] = m_b' . X_{slot i}[s]
        ps_q = psum.tile([BC, 2, 512], f32, tag="big", bufs=2)
        nslices = [(j * 512, min(512, qn - j * 512)) for j in range((qn + 511) // 512)]
        for (j0, jn) in nslices:
            for c in range(C6):
                nc.tensor.matmul(
                    ps_q[:, j0 // 512, 0:jn],
                    mt_sb[:, c, :],
                    xtq_sb[q][:, c, j0 : j0 + jn],
                    start=(c == 0),
                    stop=(c == C6 - 1),
                )
        # diagonal extraction: zero off-diagonal blocks, reduce over i
        masked = work.tile([BC, QB, lq], f32, tag="masked", bufs=2)
        nc.vector.tensor_tensor(
            out=masked[:],
            in0=ps_q[:].rearrange("p h n -> p (h n)")[:, 0:qn].rearrange(
                "p (i s) -> p i s", i=QB
            ),
            in1=dmq_sb[q][:],
            op=mybir.AluOpType.mult,
        )
        red = work.tile([BC, L], f32, tag="red", bufs=2)
        nc.vector.memset(red[:], 0.0)
        nc.vector.tensor_reduce(
            out=red[:, 0:lq],
            in_=masked[:].rearrange("p i s -> p s i"),
            axis=mybir.AxisListType.X,
            op=mybir.AluOpType.add,
        )
        # masked softmax over the free dim (rows outside this quarter are
        # garbage and never read downstream)
        scores_sb = work.tile([BC, L], f32, tag="scores_sb", bufs=2)
        nc.vector.tensor_tensor(
            out=scores_sb[:], in0=red[:], in1=amask_sb[:], op=mybir.AluOpType.add
        )
        mx = work.tile([BC, 1], f32, tag="mx", bufs=2)
        nc.vector.tensor_reduce(
            out=mx[:],
            in_=scores_sb[:],
            axis=mybir.AxisListType.X,
            op=mybir.AluOpType.max,
        )
        negmax = work.tile([BC, 1], f32, tag="negmax", bufs=2)
        nc.vector.tensor_scalar_mul(negmax[:], mx[:], -1.0)
        p_sb = work.tile([BC, L], f32, tag="p_sb", bufs=2)
        sumexp = work.tile([BC, 1], f32, tag="sumexp", bufs=2)
        nc.scalar.activation(
            out=p_sb[:],
            in_=scores_sb[:],
            func=mybir.ActivationFunctionType.Exp,
            bias=negmax[:],
            scale=1.0,
            accum_out=sumexp[:],
        )
        rsum = work.tile([BC, 1], f32, tag="rsum", bufs=2)
        nc.vector.reciprocal(out=rsum[:], in_=sumexp[:])
        nc.vector.tensor_scalar_mul(p_sb[:], p_sb[:], rsum[:])

        # p^T for this quarter (cast bf16)
        ps_pt = psum.tile([L, BC], f32, tag="ptq", bufs=2)
        nc.tensor.transpose(ps_pt[:], p_sb[:], ident_f[:])
        pt_sb = work.tile([L, BC], bf16, tag="pt_sb", bufs=2)
        nc.vector.tensor_copy(pt_sb[:], ps_pt[:])

        # att^T[d, b] for the 8 slots of this quarter
        for i in range(QB):
            b = q * QB + i
            for c in range(C6):
                nc.tensor.matmul(
                    ps_attt[:, c, b : b + 1],
                    xq_sb[q][:, i, c * 128 : (c + 1) * 128],  # lhsT [lq, 128]
                    pt_sb[0:lq, b : b + 1],                   # rhs  [lq, 1]
                    start=True,
                    stop=True,
                )

    attt_sb = work.tile([128, C6, BC], bf16, tag="attt_sb")
    nc.vector.tensor_copy(attt_sb[:], ps_attt[:])

    # ---- dense: h^T[o, b] = tanh(sum_i W[o,i] feats[b,i] + db[o]) --------
    ps_ht = psum.tile([128, C6, BC], f32, tag="ht")
    for c in range(C6):
        for k in range(C12):
            rhs = attt_sb[:, k, :] if k < C6 else mtb_sb[:, k - C6, :]
            nc.tensor.matmul(
                ps_ht[:, c, :],
                wdt_sb[:, k, c * 128 : (c + 1) * 128],
                rhs,
                start=(k == 0),
                stop=(k == C12 - 1),
            )
    ht_sb = work.tile([128, C6, BC], bf16, tag="ht_sb")
    for c in range(C6):
        nc.scalar.activation(
            out=ht_sb[:, c, :],
            in_=ps_ht[:, c, :],
            func=mybir.ActivationFunctionType.Tanh,
            bias=db_sb[:, c : c + 1],
            scale=1.0,
        )

    # ---- decoder (21 label rows) -> label mixing -------------------------
    ps_p21 = psum.tile([NK, BC], f32, tag="ptq", bufs=2)
    for c in range(C6):
        nc.tensor.matmul(
            ps_p21[:],
            dect_sb[:, c, :],
            ht_sb[:, c, :],
            start=(c == 0),
            stop=(c == C6 - 1),
        )
    p21t_sb = work.tile([NK, BC], f32, tag="p21t_sb")
    nc.scalar.activation(
        out=p21t_sb[:],
        in_=ps_p21[:],
        func=mybir.ActivationFunctionType.Tanh,
        bias=dbdec_sb[:],
        scale=1.0,
    )
    ps_out6 = psum.tile([BC, 6], f32, tag="ptq", bufs=2)
    nc.tensor.matmul(ps_out6[:], p21t_sb[:], wf_sb[:], start=True, stop=True)
    out6_sb = work.tile([BC, 6], f32, tag="out6_sb")
    nc.vector.tensor_copy(out6_sb[:], ps_out6[:])
    nc.sync.dma_start(out=t["out6"][:], in_=out6_sb[:])

    ctx.close()


def _build(lqs):
    key = tuple(lqs)
    if key in _cache:
        return _cache[key]
    f32 = mybir.dt.float32
    bf16 = mybir.dt.bfloat16
    nc = bacc.Bacc("TRN2", target_bir_lowering=False, debug=False)
    t = {}

    def din(name, shape, dt):
        t[name] = nc.dram_tensor(name, shape, dt, kind="ExternalInput").ap()

    def dout(name, shape, dt):
        t[name] = nc.dram_tensor(name, shape, dt, kind="ExternalOutput").ap()

    f16 = mybir.dt.float16
    nxt = C6 * BC + sum(C6 * QB * lq for lq in lqs)
    ndm = sum(QB * lq for lq in lqs)
    din("xtall", [128, nxt], f16)
    din("dmall", [BC, ndm], f32)
    din("wpack", [128, C12 * D + PB], bf16)
    din("packf", [128, PF], f32)
    for q, lq in enumerate(lqs):
        din(f"xq{q}", [lq, QB, D], bf16)
    dout("out6", [BC, 6], f32)
    dout("catt", [2, BC], f32)

    with tile.TileContext(nc) as tc:
        _body(nc, tc, t, lqs)
    nc.compile()
    _cache[key] = (nc, t)
    return _cache[key]


def _chunkT(a2d):
    """[N, D-like] -> [128, D//128, N] chunked transpose layout."""
    d = a2d.shape[1]
    return np.ascontiguousarray(
        a2d.T.reshape(d // 128, 128, a2d.shape[0]).transpose(1, 0, 2)
    )


def kernel(**inputs):
    global LAST_RESULT
    bert = np.asarray(inputs["bert_out"], dtype=np.float32)      # [B, S, D]
    ids = np.asarray(inputs["input_ids"])
    length = np.asarray(inputs["length"]).astype(np.int64)
    senti_w = np.asarray(inputs["senti_w"], dtype=np.float32)
    senti_b = np.asarray(inputs["senti_b"], dtype=np.float32)
    dense_w = np.asarray(inputs["dense_w"], dtype=np.float32)    # [D, 2D]
    dense_b = np.asarray(inputs["dense_b"], dtype=np.float32)
    dec_w = np.asarray(inputs["dec_w"], dtype=np.float32)        # [V, D]
    dec_b = np.asarray(inputs["dec_b"], dtype=np.float32)
    w0 = np.asarray(inputs["w0"], dtype=np.float32)
    w1 = np.asarray(inputs["w1"], dtype=np.float32)
    w2 = np.asarray(inputs["w2"], dtype=np.float32)

    mask_pos = np.argmax(ids == MASK_ID, axis=1)                 # [B]
    m = bert[np.arange(B), mask_pos]                             # [B, D]
    pooler = bert[:, 0]                                          # [B, D]

    # global sort by length; rank r -> core r%8, slot r//8
    order = np.argsort(length, kind="stable")
    lens = length[order]
    lqs = []
    for q in range(NQ):
        lq = int(lens[(q + 1) * QB * NCORES - 1])
        lqs.append(min(L, ((lq + 7) // 8) * 8))

    # shared weight layouts
    wdt = np.ascontiguousarray(
        dense_w.T.reshape(C12, 128, D).transpose(1, 0, 2)
    ).astype(BF16)
    dec21 = dec_w[FLAT_IDS]
    dect = _chunkT(dec21).astype(BF16)
    sentit = _chunkT(senti_w)
    wf = np.zeros((NK, 6), np.float32)
    off = 0
    for g, wg in enumerate((w0, w1, w2)):
        k = wg.shape[1]
        for j in range(2):
            wf[off : off + k, j * 3 + g] = wg[j]
        off += k
    db = np.ascontiguousarray(dense_b.reshape(C6, 128).T)
    dbdec = dec_b[FLAT_IDS][:, None].astype(np.float32)
    sb = senti_b[:, None]

    nc, t = _build(lqs)

    in_maps = []
    core_rows = []
    for k in range(NCORES):
        rows = order[np.arange(BC) * NCORES + k]                 # 32 batch idx
        core_rows.append(rows)
        rlen = length[rows]
        im = {}
        mtk = _chunkT(m[rows])
        xt_parts = [mtk.astype(np.float16).reshape(128, C6 * BC)]
        dm_parts = []
        for q, lq in enumerate(lqs):
            r = rows[q * QB : (q + 1) * QB]
            rl = rlen[q * QB : (q + 1) * QB]
            padded = np.zeros((QB, lq, D), np.float32)
            for i in range(QB):
                padded[i, : rl[i]] = bert[r[i], 3 : 3 + rl[i]]
            im[f"xq{q}"] = np.ascontiguousarray(
                padded.transpose(1, 0, 2)
            ).astype(BF16)
            xt_parts.append(
                np.ascontiguousarray(
                    padded.transpose(2, 0, 1)
                    .reshape(C6, 128, QB, lq)
                    .transpose(1, 0, 2, 3)
                )
                .reshape(128, C6 * QB * lq)
                .astype(np.float16)
            )
            dmq = np.zeros((BC, QB, lq), np.float32)
            for i in range(QB):
                dmq[q * QB + i, i, :] = 1.0
            dm_parts.append(dmq.reshape(BC, QB * lq))
        im["xtall"] = np.concatenate(xt_parts, axis=1)
        im["dmall"] = np.concatenate(dm_parts, axis=1)
        amask = np.where(
            np.arange(L)[None, :] < rlen[:, None], 0.0, -1e30
        ).astype(np.float32)
        packf = np.zeros((128, PF), np.float32)
        packf[:, 0:192] = mtk.reshape(128, 192)
        packf[:, 192:384] = _chunkT(pooler[rows]).reshape(128, 192)
        packf[:, 384:396] = sentit.reshape(128, 12)
        packf[:, 396:402] = db
        packf[0:NK, 402:408] = wf
        packf[0:NK, 408:409] = dbdec
        packf[0:2, 409:410] = sb
        packf[0:BC, 410 : 410 + L] = amask
        wpack = np.zeros((128, C12 * D + PB), BF16)
        wpack[:, 0 : C12 * D] = wdt.reshape(128, C12 * D)
        wpack[:, C12 * D : C12 * D + 192] = mtk.astype(BF16).reshape(128, 192)
        wpack[:, C12 * D + 192 :] = dect.reshape(128, C6 * NK)
        im["wpack"] = wpack
        im["packf"] = packf
        in_maps.append(im)

    res = bass_utils.run_bass_kernel_spmd(
        nc, in_maps, core_ids=list(range(NCORES)), trace=KERNEL_TRACE
    )
    LAST_RESULT = res

    category_out = np.empty((B, 2), np.float32)
    out = np.empty((B, 2, 3), np.float32)
    for k in range(NCORES):
        rows = core_rows[k]
        category_out[rows] = res.results[k]["catt"].T
        out[rows] = res.results[k]["out6"].reshape(BC, 2, 3)
    return category_out, out
